# revision 2
# baseline (speedup 1.0000x reference)
"""MiniS4D Trainium2 kernel v2 (8 NeuronCores, batch-data-parallel).

T=32 chunking with 4-channel block-diagonal matmuls (256-col streams),
single-instruction prefix scans (tensor_tensor_scan), SBUF->SBUF DMA
partition-gather for the t-major -> c-major y reassembly (no DMA-XBAR
transposes, no DRAM roundtrip), host-side u permute to the block layout.

Per group g of 4 channels (c = 4g+q), partition p = 32q + t':
  intra: y[(q,t),(b,m)] = sum_{t'<=t} keff32[c,t-t'] u[b,c,32m+t']   (BD toep)
  Q[(q,nh),(b,m)]       = Re/Im sum_{t'} r^{t'} u[b,c,32m+t']        (BD v2)
  Qt = E (.) Q  (complex, E[m] = e^m, e = r^32)   [DVE shuffle+mul+add]
  PP = cumsum_m Qt                                [tensor_tensor_scan]
  Hs[m] = K[m] (.) PP[m-1]  (K[m] = r^{L-1-32m})  [DVE]
  y += v1BD^T Hs                                  (BD v1, same PSUM)
GELU -> gather to c-major Ybig -> W_out mix -> GLU -> mean -> dec.
"""
import sys
sys.path.insert(0, "/opt/trn_rl_repo")
import numpy as np

import concourse.bass as bass
import concourse.tile as tile
from concourse import bacc, mybir
from concourse import bass_utils
from concourse.bass import broadcast_tensor_aps

F32 = mybir.dt.float32
F16 = mybir.dt.float16
AF = mybir.ActivationFunctionType
ALU = mybir.AluOpType
GELU_FUNC = None  # set to AF.Copy for CoreSim (no Gelu there)
DEBUG_TAPS = False

B, C, L, N = 16, 512, 4096, 8
T, M, G = 32, 128, 128
NW = 16            # waves of 8 groups
NCORES, BL = 8, 2

SHUF = [(i % 16 + 8) % 16 + 16 * (i // 16) for i in range(32)]

_compiled = None


def _prep(inputs):
    """Host-side parameter preparation (numpy, float64 internally)."""
    log_dt = inputs["log_dt"].astype(np.float64)
    A = -np.exp(inputs["log_A_real"].astype(np.float64)) \
        + 1j * inputs["A_imag"].astype(np.float64)
    dt = np.exp(log_dt)
    r = np.exp(dt[:, None] * A)                                   # (C, N)
    Bc = inputs["B_re"].astype(np.float64) + 1j * inputs["B_im"].astype(np.float64)
    Cc = inputs["C_re"].astype(np.float64) + 1j * inputs["C_im"].astype(np.float64)
    w = Cc * (r - 1.0) / A * Bc                                   # (C, N)
    rinv = 1.0 / r
    wL = w * r ** (L - 1)

    lags = np.arange(T)
    keff32 = np.real(wL[:, :, None] * rinv[:, :, None] ** lags).sum(1)  # (C, 32)
    keff32[:, 0] += inputs["D"].astype(np.float64)

    gidx = np.arange(G)
    toepbd = np.zeros((G, 128, 128), np.float16)
    v2bd = np.zeros((G, 128, 64), np.float16)
    v1bd = np.zeros((G, 64, 128), np.float16)
    rp = r[:, None, :] ** lags[:, None]                           # (C, 32, N)
    wr = w[:, None, :] * rinv[:, None, :] ** lags[:, None]        # (C, 32, N)
    for q in range(4):
        cs = 4 * gidx + q
        blk = np.zeros((G, 32, 32), np.float64)
        for d in range(T):
            idx = np.arange(T - d)
            blk[:, idx, idx + d] = keff32[cs, d][:, None]
        toepbd[:, 32 * q:32 * q + 32, 32 * q:32 * q + 32] = blk.astype(np.float16)
        v2bd[:, 32 * q:32 * q + 32, 16 * q:16 * q + 8] = rp[cs].real
        v2bd[:, 32 * q:32 * q + 32, 16 * q + 8:16 * q + 16] = rp[cs].imag
        v1bd[:, 16 * q:16 * q + 8, 32 * q:32 * q + 32] = \
            wr[cs].real.transpose(0, 2, 1)
        v1bd[:, 16 * q + 8:16 * q + 16, 32 * q:32 * q + 32] = \
            -wr[cs].imag.transpose(0, 2, 1)

    ms = np.arange(M)
    Em = (r ** T)[:, :, None] ** ms                               # (C, N, M)
    Km = r[:, :, None] ** (L - 1 - T * ms)                        # (C, N, M)
    E1 = np.concatenate([Em.real, Em.real], 1).astype(np.float16)  # (C,16,M)
    E2 = np.concatenate([-Em.imag, Em.imag], 1).astype(np.float16)
    K1 = np.concatenate([Km.real, Km.real], 1).astype(np.float16)
    K2 = np.concatenate([-Km.imag, Km.imag], 1).astype(np.float16)

    # esc[w, p=64*gh+16*q+nh, a, j, m], c = 4*(8w+2j+gh)+q
    esc = np.zeros((NW, 128, 4, 4, M), np.float16)
    warr = np.arange(NW)[:, None]
    jarr = np.arange(4)[None, :]
    for gh in range(2):
        for q in range(4):
            cs = 4 * (8 * warr + 2 * jarr + gh) + q               # (NW, 4)
            p0 = 64 * gh + 16 * q
            for a, S in enumerate((E1, E2, K1, K2)):
                esc[:, p0:p0 + 16, a, :, :] = S[cs].transpose(0, 2, 1, 3)

    wmix = np.ascontiguousarray(
        inputs["W_out"].T.astype(np.float16).reshape(4, 128, 1024))
    b_out = inputs["b_out"].astype(np.float32)
    bouta = np.ascontiguousarray(b_out[:512].reshape(4, 128).T)   # (128, 4)
    boutg = np.ascontiguousarray(b_out[512:].reshape(4, 128).T)
    wdecp = np.ascontiguousarray(
        (inputs["W_dec"][0].astype(np.float32) / L).reshape(4, 128).T)  # (128,4)
    bdec = inputs["b_dec"].astype(np.float32).reshape(1, 1)

    aux = dict(toepbd=toepbd, v2bd=v2bd, v1bd=v1bd, esc=esc, wmix=wmix,
               bouta=bouta, boutg=boutg, wdecp=wdecp, bdec=bdec)

    # u_bd per core: [G, 128(32q+t'), BL, M]
    u16 = inputs["u"].astype(np.float16)                          # (B, C, L)
    u_bds = []
    for k in range(NCORES):
        ub = u16[BL * k:BL * k + BL].reshape(BL, G, 4, M, T)
        u_bds.append(np.ascontiguousarray(
            ub.transpose(1, 2, 4, 0, 3).reshape(G, 128, BL, M)))
    return aux, u_bds


def _build():
    nc = bacc.Bacc("TRN2", target_bir_lowering=False, debug=False,
                   num_devices=NCORES)
    d_u = nc.dram_tensor("u_bd", [G, 128, BL, M], F16, kind="ExternalInput").ap()
    d_toep = nc.dram_tensor("toepbd", [G, 128, 128], F16, kind="ExternalInput").ap()
    d_v2 = nc.dram_tensor("v2bd", [G, 128, 64], F16, kind="ExternalInput").ap()
    d_v1 = nc.dram_tensor("v1bd", [G, 64, 128], F16, kind="ExternalInput").ap()
    d_esc = nc.dram_tensor("esc", [NW, 128, 4, 4, M], F16, kind="ExternalInput").ap()
    d_wmix = nc.dram_tensor("wmix", [4, 128, 1024], F16, kind="ExternalInput").ap()
    d_bouta = nc.dram_tensor("bouta", [128, 4], F32, kind="ExternalInput").ap()
    d_boutg = nc.dram_tensor("boutg", [128, 4], F32, kind="ExternalInput").ap()
    d_wdecp = nc.dram_tensor("wdecp", [128, 4], F32, kind="ExternalInput").ap()
    d_bdec = nc.dram_tensor("bdec", [1, 1], F32, kind="ExternalInput").ap()
    d_out = nc.dram_tensor("odec", [1, 2], F32, kind="ExternalOutput").ap()
    if DEBUG_TAPS:
        d_dbg_qs = nc.dram_tensor("dbg_qs", [128, 4, BL, M], F16, kind="ExternalOutput").ap()
        d_dbg_qt = nc.dram_tensor("dbg_qt", [128, 4, BL, M], F16, kind="ExternalOutput").ap()
        d_dbg_pp = nc.dram_tensor("dbg_pp", [128, 4, BL, M + 1], F16, kind="ExternalOutput").ap()
        d_dbg_ht = nc.dram_tensor("dbg_ht", [128, 4, BL, M], F16, kind="ExternalOutput").ap()
        d_dbg_y = nc.dram_tensor("dbg_y", [128, 4, 32, BL, M], F16, kind="ExternalOutput").ap()
        d_dbg_m1 = nc.dram_tensor("dbg_m1", [128, 4, BL, 16], F32, kind="ExternalOutput").ap()

    with tile.TileContext(nc) as tc:
        with tc.tile_pool(name="const", bufs=1) as constp:
            bouta_sb = constp.tile([128, 4], F32)
            nc.sync.dma_start(bouta_sb[:], d_bouta[:])
            boutg_sb = constp.tile([128, 4], F32)
            nc.sync.dma_start(boutg_sb[:], d_boutg[:])
            wdec_sb = constp.tile([128, 4], F32)
            nc.sync.dma_start(wdec_sb[:], d_wdecp[:])
            bdec_sb = constp.tile([1, 1], F32)
            nc.sync.dma_start(bdec_sb[:], d_bdec[:])
            zero_sb = constp.tile([128, M], F16)
            nc.vector.memset(zero_sb[:], 0.0)
            ybig = constp.tile([128, 4, 32, BL, M], F16)
            wm = []
            for i in range(4):
                wt = constp.tile([128, 1024], F16, tag=f"wm{i}")
                nc.scalar.dma_start(wt[:], d_wmix[i])
                wm.append(wt)

            # ================= SSM phase (16 waves of 8 groups) =========
            with tc.tile_pool(name="ubp", bufs=18) as ubp, \
                 tc.tile_pool(name="v2p", bufs=6) as v2p, \
                 tc.tile_pool(name="tpp", bufs=18) as tpp, \
                 tc.tile_pool(name="v1p", bufs=18) as v1p, \
                 tc.tile_pool(name="escp", bufs=2) as escp, \
                 tc.tile_pool(name="stp", bufs=2) as stp, \
                 tc.tile_pool(name="ygp", bufs=6) as ygp, \
                 tc.tile_pool(name="qps", bufs=2, space="PSUM") as qpsp, \
                 tc.tile_pool(name="yps", bufs=4, space="PSUM") as ypsp:

                ub_t = {}
                tp_t = {}
                v1_t = {}
                ht_t = {}

                def v2_stage(w):
                    esct = escp.tile([128, 4, 4, M], F16)
                    nc.scalar.dma_start(esct[:], d_esc[w])
                    qps = qpsp.tile([128, 4, BL, M], F32)
                    for jg in range(8):
                        g = 8 * w + jg
                        ub = ubp.tile([128, BL, M], F16, tag="ub")
                        nc.scalar.dma_start(ub[:], d_u[g])
                        ub_t[g] = ub
                        v2t = v2p.tile([128, 64], F16, tag="v2")
                        nc.scalar.dma_start(v2t[:], d_v2[g])
                        tpt = tpp.tile([128, 128], F16, tag="tp")
                        nc.scalar.dma_start(tpt[:], d_toep[g])
                        tp_t[g] = tpt
                        j, gh = jg // 2, jg % 2
                        v1t = v1p.tile([128, 128], F16, tag="v1")
                        nc.scalar.dma_start(v1t[64 * gh:64 * gh + 64, :],
                                            d_v1[g])
                        v1_t[g] = v1t
                        nc.tensor.matmul(
                            qps[64 * gh:64 * gh + 64, j:j + 1].squeeze(),
                            v2t[:], ub[:],
                            start=True, stop=True, tile_position=(0, 64 * gh))
                    qs = stp.tile([128, 4, BL, M], F16, tag="qs")
                    nc.scalar.activation(qs[:], qps[:], AF.Copy)
                    sh = stp.tile([128, 4, BL, M], F16, tag="sh")
                    nc.vector.stream_shuffle(sh[:], qs[:], SHUF)
                    t1 = stp.tile([128, 4, BL, M], F16, tag="t1")

                    def bmul(dst, src, a):
                        sl = esct[:, a:a + 1, :, :].squeeze().unsqueeze(2)
                        _, e_ap = broadcast_tensor_aps(src, sl)
                        nc.vector.tensor_mul(dst, src, e_ap)

                    bmul(t1[:], qs[:], 0)
                    bmul(sh[:], sh[:], 1)
                    qt = stp.tile([128, 4, BL, M], F16, tag="qt")
                    nc.vector.tensor_add(qt[:], t1[:], sh[:])
                    pps = stp.tile([128, 4, BL, M + 1], F16, tag="pps")
                    nc.vector.memset(pps[:, :, :, 0:1], 0.0)
                    for j in range(4):
                        for b in range(BL):
                            nc.vector.tensor_tensor_scan(
                                pps[:, j:j + 1, b:b + 1, 1:M + 1].squeeze(),
                                qt[:, j:j + 1, b:b + 1, :].squeeze(),
                                zero_sb[:], 0.0, ALU.add, ALU.add)
                    psh = stp.tile([128, 4, BL, M + 1], F16, tag="psh")
                    nc.vector.stream_shuffle(psh[:, :, :, 0:M],
                                             pps[:, :, :, 0:M], SHUF)
                    bmul(t1[:], pps[:, :, :, 0:M], 2)
                    bmul(psh[:, :, :, 0:M], psh[:, :, :, 0:M], 3)
                    ht = stp.tile([128, 4, BL, M], F16, tag="ht")
                    nc.vector.tensor_add(ht[:], t1[:], psh[:, :, :, 0:M])
                    ht_t[w] = ht
                    if DEBUG_TAPS and w == 0:
                        nc.sync.dma_start(d_dbg_qs[:], qs[:])
                        nc.sync.dma_start(d_dbg_qt[:], qt[:])
                        nc.sync.dma_start(d_dbg_pp[:], pps[:])
                        nc.sync.dma_start(d_dbg_ht[:], ht[:])

                def conv_stage(w):
                    ht = ht_t.pop(w)
                    for jg in range(8):
                        g = 8 * w + jg
                        j, gh = jg // 2, jg % 2
                        yps = ypsp.tile([128, BL, M], F32)
                        nc.tensor.matmul(yps[:], tp_t.pop(g)[:], ub_t[g][:],
                                         start=True, stop=False)
                        nc.tensor.matmul(
                            yps[:],
                            v1_t.pop(g)[64 * gh:64 * gh + 64, :],
                            ht[64 * gh:64 * gh + 64, j:j + 1].squeeze(),
                            start=False, stop=True,
                            tile_position=(64 * gh, 0))
                        ub_t.pop(g)
                        yg = ygp.tile([128, BL, M], F16, tag="yg")
                        nc.scalar.activation(yg[:], yps[:], GELU_FUNC or AF.Gelu)
                        ghat, i = g % 32, g // 32
                        nc.sync.dma_start(
                            ybig[4 * ghat:4 * ghat + 4, i:i + 1].squeeze(),
                            yg[:])

                for w in range(NW + 1):
                    if w < NW:
                        v2_stage(w)
                    if w >= 1:
                        conv_stage(w - 1)

            if DEBUG_TAPS:
                nc.sync.dma_start(d_dbg_y[:], ybig[:])
            # ================= Mix phase ================================
            with tc.tile_pool(name="m1p", bufs=1) as m1pool:
                m1 = m1pool.tile([128, 4, BL, 16], F32)
                with tc.tile_pool(name="sgp", bufs=4) as sgp, \
                     tc.tile_pool(name="zps", bufs=3, space="PSUM") as zpsp:
                    for j in range(4):
                        for ct in range(16):
                            za = zpsp.tile([128, 2, BL, M], F32, tag="za")
                            zg = zpsp.tile([128, 2, BL, M], F32, tag="zg")
                            for side, zt in ((0, za), (1, zg)):
                                o = j + 4 * side
                                for i in range(4):
                                    nc.tensor.matmul(
                                        zt[:],
                                        wm[i][:, 128 * o:128 * o + 128],
                                        ybig[:, i:i + 1,
                                             2 * ct:2 * ct + 2].squeeze(),
                                        start=(i == 0), stop=(i == 3))
                            sg = sgp.tile([128, 2, BL, M], F16, tag="sg")
                            nc.scalar.activation(sg[:], zg[:], AF.Sigmoid,
                                                 bias=boutg_sb[:, j:j + 1])
                            for b in range(BL):
                                scr = sgp.tile([128, 2, M], F16, tag="scr")
                                nc.vector.scalar_tensor_tensor(
                                    scr[:],
                                    za[:, :, b:b + 1, :].squeeze(),
                                    bouta_sb[:, j:j + 1],
                                    sg[:, :, b:b + 1, :].squeeze(),
                                    op0=ALU.add, op1=ALU.mult,
                                    accum_out=m1[:, j:j + 1, b:b + 1,
                                                 ct:ct + 1].squeeze()
                                    .unsqueeze(1))

                if DEBUG_TAPS:
                    nc.sync.dma_start(d_dbg_m1[:], m1[:])
                # ---- decode ----
                with tc.tile_pool(name="dps", bufs=1, space="PSUM") as dpsp:
                    r1 = m1pool.tile([128, 4, BL], F32)
                    nc.vector.reduce_sum(r1[:], m1[:], axis=mybir.AxisListType.X)
                    r2 = m1pool.tile([128, BL, 4], F32)
                    for b in range(BL):
                        nc.vector.tensor_mul(
                            r2[:, b:b + 1, :].squeeze(),
                            r1[:, :, b:b + 1].squeeze(), wdec_sb[:])
                    r3 = m1pool.tile([128, BL], F32)
                    nc.vector.reduce_sum(r3[:], r2[:], axis=mybir.AxisListType.X)
                    ones = m1pool.tile([128, 1], F32)
                    nc.vector.memset(ones[:], 1.0)
                    dp = dpsp.tile([1, 2], F32)
                    nc.tensor.matmul(dp[:], ones[:], r3[:], start=True, stop=True)
                    osb = m1pool.tile([1, 2], F32)
                    nc.vector.tensor_scalar_add(osb[:], dp[:], bdec_sb[:, 0:1])
                    nc.sync.dma_start(d_out[:], osb[:])

    nc.compile()
    return nc


def _get_compiled():
    global _compiled
    if _compiled is None:
        _compiled = _build()
    return _compiled


def _run(inputs, trace=False, **kw):
    aux, u_bds = _prep(inputs)
    nc = _get_compiled()
    in_maps = []
    for cid in range(NCORES):
        m = dict(aux)
        m["u_bd"] = u_bds[cid]
        in_maps.append(m)
    return bass_utils.run_bass_kernel_spmd(
        nc, in_maps, core_ids=list(range(NCORES)), trace=trace, **kw)


def kernel(**inputs):
    inputs = {k: np.asarray(v) for k, v in inputs.items()}
    res = _run(inputs)
    out = np.empty((B, 1), np.float32)
    for cid in range(NCORES):
        out[BL * cid:BL * cid + BL, 0] = res.results[cid]["odec"][0, :]
    return out


# revision 3
# speedup vs baseline: 1.9402x; 1.9402x over previous
"""MiniS4D Trainium2 kernel v3 (8 NeuronCores, batch-data-parallel).

v2 -> v3: wave-batched DMA loads issued from the Sync engine (the v2
Activation engine spent 321us dispatching 528 per-group DMAs), single
masked tensor_tensor_scan per wave (was 8), fp8(e4m3) y + DoubleRow
fp8 mix matmuls (2x PE throughput, K=256 per instruction).

Structure per group g of 4 channels (c = 4g+q), partition p = 32q+t':
  intra: y[(q,t),(b,m)] = sum_{t'<=t} keff32[c,t-t'] u[b,c,32m+t']   (BD toep)
  Q[(q,nh),(b,m)]       = Re/Im sum_{t'} r^{t'} u[b,c,32m+t']        (BD v2)
  Qt = E (.) Q  (complex, E[m] = e^m, e = r^32)   [DVE shuffle+mul+add]
  PP = segmented-cumsum_m Qt  (mask resets at m=0) [one tensor_tensor_scan]
  Hs[m] = K[m] (.) PP[m-1]  (K[m] = r^{L-1-32m}, K[0] := 0)  [DVE]
  y += v1BD^T Hs                                  (BD v1, same PSUM)
GELU -> fp8 -> SBUF-to-SBUF DMA gather to c-major Ybig ->
DoubleRow fp8 W_out mix (W pre-scaled by 32) -> GLU -> mean -> dec.
"""
import sys
sys.path.insert(0, "/opt/trn_rl_repo")
import numpy as np
import ml_dtypes

import concourse.bass as bass
import concourse.tile as tile
from concourse import bacc, mybir
from concourse import bass_utils
from concourse.bass import broadcast_tensor_aps

F32 = mybir.dt.float32
F16 = mybir.dt.float16
F8 = mybir.dt.float8e4
NP8 = ml_dtypes.float8_e4m3fn
AF = mybir.ActivationFunctionType
ALU = mybir.AluOpType
PM = mybir.MatmulPerfMode
GELU_FUNC = None   # set to AF.Copy for CoreSim (no Gelu there)
DEBUG_TAPS = False
WSCALE = 32.0      # fp8 weight pre-scale

B, C, L, N = 16, 512, 4096, 8
T, M, G = 32, 128, 128
NW = 16            # waves of 8 groups
NCORES, BL = 8, 2

SHUF = [(i % 16 + 8) % 16 + 16 * (i // 16) for i in range(32)]

_compiled = None


def _prep(inputs):
    """Host-side parameter preparation (numpy, float64 internally)."""
    log_dt = inputs["log_dt"].astype(np.float64)
    A = -np.exp(inputs["log_A_real"].astype(np.float64)) \
        + 1j * inputs["A_imag"].astype(np.float64)
    dt = np.exp(log_dt)
    r = np.exp(dt[:, None] * A)                                   # (C, N)
    Bc = inputs["B_re"].astype(np.float64) + 1j * inputs["B_im"].astype(np.float64)
    Cc = inputs["C_re"].astype(np.float64) + 1j * inputs["C_im"].astype(np.float64)
    w = Cc * (r - 1.0) / A * Bc                                   # (C, N)
    rinv = 1.0 / r
    wL = w * r ** (L - 1)

    lags = np.arange(T)
    keff32 = np.real(wL[:, :, None] * rinv[:, :, None] ** lags).sum(1)  # (C, 32)
    keff32[:, 0] += inputs["D"].astype(np.float64)

    gidx = np.arange(G)
    toepbd = np.zeros((G, 128, 128), np.float16)
    v2bd = np.zeros((G, 128, 64), np.float16)
    v1bd = np.zeros((G, 64, 128), np.float16)
    rp = r[:, None, :] ** lags[:, None]                           # (C, 32, N)
    wr = w[:, None, :] * rinv[:, None, :] ** lags[:, None]        # (C, 32, N)
    for q in range(4):
        cs = 4 * gidx + q
        blk = np.zeros((G, 32, 32), np.float64)
        for d in range(T):
            idx = np.arange(T - d)
            blk[:, idx, idx + d] = keff32[cs, d][:, None]
        toepbd[:, 32 * q:32 * q + 32, 32 * q:32 * q + 32] = blk.astype(np.float16)
        v2bd[:, 32 * q:32 * q + 32, 16 * q:16 * q + 8] = rp[cs].real
        v2bd[:, 32 * q:32 * q + 32, 16 * q + 8:16 * q + 16] = rp[cs].imag
        v1bd[:, 16 * q:16 * q + 8, 32 * q:32 * q + 32] = \
            wr[cs].real.transpose(0, 2, 1)
        v1bd[:, 16 * q + 8:16 * q + 16, 32 * q:32 * q + 32] = \
            -wr[cs].imag.transpose(0, 2, 1)

    ms = np.arange(M)
    Em = (r ** T)[:, :, None] ** ms                               # (C, N, M)
    Km = r[:, :, None] ** (L - 1 - T * ms)                        # (C, N, M)
    Km[:, :, 0] = 0.0          # Hs[0] = 0 via scale (flat-scan shift trick)
    E1 = np.concatenate([Em.real, Em.real], 1).astype(np.float16)  # (C,16,M)
    E2 = np.concatenate([-Em.imag, Em.imag], 1).astype(np.float16)
    K1 = np.concatenate([Km.real, Km.real], 1).astype(np.float16)
    K2 = np.concatenate([-Km.imag, Km.imag], 1).astype(np.float16)

    # esc[w, p=64*gh+16*q+nh, a, j, m], c = 4*(8w+2j+gh)+q
    esc = np.zeros((NW, 128, 4, 4, M), np.float16)
    warr = np.arange(NW)[:, None]
    jarr = np.arange(4)[None, :]
    for gh in range(2):
        for q in range(4):
            cs = 4 * (8 * warr + 2 * jarr + gh) + q               # (NW, 4)
            p0 = 64 * gh + 16 * q
            for a, S in enumerate((E1, E2, K1, K2)):
                esc[:, p0:p0 + 16, a, :, :] = S[cs].transpose(0, 2, 1, 3)

    # fp8 DoubleRow mix weights: w8[ip, p, i2, o] = WSCALE*W_out[o, 128*(2ip+i2)+p]
    wt = (inputs["W_out"].T.astype(np.float64) * WSCALE).reshape(2, 2, 128, 1024)
    w8 = np.ascontiguousarray(wt.transpose(0, 2, 1, 3)).astype(NP8)  # (2,128,2,1024)
    b_out = inputs["b_out"].astype(np.float32)
    bouta = np.ascontiguousarray(b_out[:512].reshape(4, 128).T) * WSCALE
    boutg = np.ascontiguousarray(b_out[512:].reshape(4, 128).T)
    wdecp = np.ascontiguousarray(
        (inputs["W_dec"][0].astype(np.float32) / (L * WSCALE)).reshape(4, 128).T)
    bdec = inputs["b_dec"].astype(np.float32).reshape(1, 1)

    aux = dict(toepbd=toepbd, v2bd=v2bd, v1bd=v1bd, esc=esc, w8=w8,
               bouta=bouta.astype(np.float32), boutg=boutg, wdecp=wdecp,
               bdec=bdec)

    # u_bd per core: [G, 128(32q+t'), BL, M]
    u16 = inputs["u"].astype(np.float16)                          # (B, C, L)
    u_bds = []
    for k in range(NCORES):
        ub = u16[BL * k:BL * k + BL].reshape(BL, G, 4, M, T)
        u_bds.append(np.ascontiguousarray(
            ub.transpose(1, 2, 4, 0, 3).reshape(G, 128, BL, M)))
    return aux, u_bds


def _build():
    nc = bacc.Bacc("TRN2", target_bir_lowering=False, debug=False,
                   num_devices=NCORES)
    d_u = nc.dram_tensor("u_bd", [G, 128, BL, M], F16, kind="ExternalInput").ap()
    d_toep = nc.dram_tensor("toepbd", [G, 128, 128], F16, kind="ExternalInput").ap()
    d_v2 = nc.dram_tensor("v2bd", [G, 128, 64], F16, kind="ExternalInput").ap()
    d_v1 = nc.dram_tensor("v1bd", [G, 64, 128], F16, kind="ExternalInput").ap()
    d_esc = nc.dram_tensor("esc", [NW, 128, 4, 4, M], F16, kind="ExternalInput").ap()
    d_w8 = nc.dram_tensor("w8", [2, 128, 2, 1024], F8, kind="ExternalInput").ap()
    d_bouta = nc.dram_tensor("bouta", [128, 4], F32, kind="ExternalInput").ap()
    d_boutg = nc.dram_tensor("boutg", [128, 4], F32, kind="ExternalInput").ap()
    d_wdecp = nc.dram_tensor("wdecp", [128, 4], F32, kind="ExternalInput").ap()
    d_bdec = nc.dram_tensor("bdec", [1, 1], F32, kind="ExternalInput").ap()
    d_out = nc.dram_tensor("odec", [1, 2], F32, kind="ExternalOutput").ap()
    if DEBUG_TAPS:
        d_dbg_ht = nc.dram_tensor("dbg_ht", [128, 4, BL, M], F16,
                                  kind="ExternalOutput").ap()
        d_dbg_y = nc.dram_tensor("dbg_y", [128, 4, 32, BL, M], F8,
                                 kind="ExternalOutput").ap()
        d_dbg_m1 = nc.dram_tensor("dbg_m1", [128, 4, BL, 8], F32,
                                  kind="ExternalOutput").ap()

    with tile.TileContext(nc) as tc:
        with tc.tile_pool(name="const", bufs=1) as constp:
            bouta_sb = constp.tile([128, 4], F32)
            nc.sync.dma_start(bouta_sb[:], d_bouta[:])
            boutg_sb = constp.tile([128, 4], F32)
            nc.sync.dma_start(boutg_sb[:], d_boutg[:])
            wdec_sb = constp.tile([128, 4], F32)
            nc.sync.dma_start(wdec_sb[:], d_wdecp[:])
            bdec_sb = constp.tile([1, 1], F32)
            nc.sync.dma_start(bdec_sb[:], d_bdec[:])
            maskt = constp.tile([128, 4, BL, M], F16)
            nc.vector.memset(maskt[:], 1.0)
            nc.vector.memset(maskt[:, :, :, 0:1], 0.0)
            ybig = constp.tile([128, 4, 32, BL, M], F8)
            w8t = []
            for ip in range(2):
                wt8 = constp.tile([128, 2, 1024], F8, tag=f"w8{ip}")
                nc.sync.dma_start(wt8[:], d_w8[ip])
                w8t.append(wt8)

            # ================= SSM phase (16 waves of 8 groups) =========
            with tc.tile_pool(name="uwp", bufs=3) as uwp, \
                 tc.tile_pool(name="twp", bufs=3) as twp, \
                 tc.tile_pool(name="vwp", bufs=3) as vwp, \
                 tc.tile_pool(name="v1p", bufs=10) as v1p, \
                 tc.tile_pool(name="escp", bufs=2) as escp, \
                 tc.tile_pool(name="stp", bufs=3) as stp, \
                 tc.tile_pool(name="ygp", bufs=8) as ygp, \
                 tc.tile_pool(name="qps", bufs=2, space="PSUM") as qpsp, \
                 tc.tile_pool(name="yps", bufs=4, space="PSUM") as ypsp:

                wave_t = {}

                def v2_stage(w):
                    g0 = 8 * w
                    esct = escp.tile([128, 4, 4, M], F16)
                    nc.sync.dma_start(esct[:], d_esc[w])
                    uw = uwp.tile([128, 8, BL, M], F16)
                    nc.sync.dma_start(uw[:],
                                      d_u[g0:g0 + 8].transpose([1, 0, 2, 3]))
                    tw = twp.tile([128, 8, 128], F16)
                    nc.sync.dma_start(tw[:],
                                      d_toep[g0:g0 + 8].transpose([1, 0, 2]))
                    vw = vwp.tile([128, 8, 64], F16)
                    nc.sync.dma_start(vw[:],
                                      d_v2[g0:g0 + 8].transpose([1, 0, 2]))
                    v1w = []
                    for j in range(4):
                        v1t = v1p.tile([128, 128], F16, tag="v1")
                        nc.sync.dma_start(v1t[:], d_v1[g0 + 2 * j:g0 + 2 * j + 2])
                        v1w.append(v1t)
                    qps = qpsp.tile([128, 4, BL, M], F32)
                    for jg in range(8):
                        j, gh = jg // 2, jg % 2
                        nc.tensor.matmul(
                            qps[64 * gh:64 * gh + 64, j:j + 1].squeeze(),
                            vw[:, jg:jg + 1, :].squeeze(),
                            uw[:, jg:jg + 1].squeeze(),
                            start=True, stop=True)
                    qs = stp.tile([128, 4, BL, M], F16, tag="qs")
                    nc.scalar.activation(qs[:], qps[:], AF.Copy)
                    sh = stp.tile([128, 4, BL, M], F16, tag="sh")
                    nc.vector.stream_shuffle(sh[:], qs[:], SHUF)
                    t1 = stp.tile([128, 4, BL, M], F16, tag="t1")

                    def bmul(dst, src, a):
                        sl = esct[:, a:a + 1, :, :].squeeze().unsqueeze(2)
                        _, e_ap = broadcast_tensor_aps(src, sl)
                        nc.vector.tensor_mul(dst, src, e_ap)

                    bmul(t1[:], qs[:], 0)
                    bmul(sh[:], sh[:], 1)
                    qt = stp.tile([128, 4, BL, M], F16, tag="qt")
                    nc.vector.tensor_add(qt[:], t1[:], sh[:])
                    # segmented flat scan: PP[jbm] with mask reset at m=0
                    ppc = stp.tile([128, 8 + 4 * BL * M], F16, tag="ppc")
                    nc.vector.memset(ppc[:, 0:8], 0.0)
                    nc.vector.tensor_tensor_scan(
                        ppc[:, 8:8 + 4 * BL * M],
                        maskt[:].rearrange("p j b m -> p (j b m)"),
                        qt[:].rearrange("p j b m -> p (j b m)"),
                        0.0, ALU.mult, ALU.add)
                    ppshift = ppc[:, 7:7 + 4 * BL * M].rearrange(
                        "p (j b m) -> p j b m", j=4, b=BL, m=M)
                    psh = stp.tile([128, 4, BL, M], F16, tag="psh")
                    nc.vector.stream_shuffle(
                        psh[:].rearrange("p j b m -> p (j b m)"),
                        ppc[:, 7:7 + 4 * BL * M], SHUF)
                    bmul(t1[:], ppshift, 2)
                    bmul(psh[:], psh[:], 3)
                    ht = stp.tile([128, 4, BL, M], F16, tag="ht")
                    nc.vector.tensor_add(ht[:], t1[:], psh[:])
                    wave_t[w] = (uw, tw, v1w, ht)
                    if DEBUG_TAPS and w == 0:
                        nc.sync.dma_start(d_dbg_ht[:], ht[:])

                def conv_stage(w):
                    uw, tw, v1w, ht = wave_t.pop(w)
                    for jg in range(8):
                        g = 8 * w + jg
                        j, gh = jg // 2, jg % 2
                        yps = ypsp.tile([128, BL, M], F32)
                        nc.tensor.matmul(yps[:],
                                         tw[:, jg:jg + 1, :].squeeze(),
                                         uw[:, jg:jg + 1].squeeze(),
                                         start=True, stop=False)
                        nc.tensor.matmul(
                            yps[:],
                            v1w[j][64 * gh:64 * gh + 64, :],
                            ht[64 * gh:64 * gh + 64, j:j + 1].squeeze(),
                            start=False, stop=True)
                        yg = ygp.tile([128, BL, M], F8, tag="yg")
                        nc.scalar.activation(yg[:], yps[:], GELU_FUNC or AF.Gelu)
                        ghat, i = g % 32, g // 32
                        nc.sync.dma_start(
                            ybig[4 * ghat:4 * ghat + 4, i:i + 1].squeeze(),
                            yg[:])

                for w in range(NW + 1):
                    if w < NW:
                        v2_stage(w)
                    if w >= 1:
                        conv_stage(w - 1)

            if DEBUG_TAPS:
                nc.sync.dma_start(d_dbg_y[:], ybig[:])

            # ============ Mix phase (fp8 DoubleRow, K=256/instr) ========
            with tc.tile_pool(name="m1p", bufs=1) as m1pool:
                m1 = m1pool.tile([128, 4, BL, 8], F32)
                with tc.tile_pool(name="sgp", bufs=4) as sgp, \
                     tc.tile_pool(name="zps", bufs=2, space="PSUM") as zpsp:
                    for j in range(4):
                        for tg in range(8):
                            za = zpsp.tile([128, 4, BL, M], F32, tag="za")
                            zg = zpsp.tile([128, 4, BL, M], F32, tag="zg")
                            for side, zt in ((0, za), (1, zg)):
                                o = j + 4 * side
                                for ip in range(2):
                                    for hw in range(2):
                                        t0 = 4 * tg + 2 * hw
                                        nc.tensor.matmul(
                                            zt[:, 2 * hw:2 * hw + 2],
                                            w8t[ip][:, :,
                                                    128 * o:128 * o + 128],
                                            ybig[:, 2 * ip:2 * ip + 2,
                                                 t0:t0 + 2],
                                            start=(ip == 0), stop=(ip == 1),
                                            perf_mode=PM.DoubleRow)
                            sg = sgp.tile([128, 4, BL, M], F16, tag="sg")
                            nc.scalar.activation(sg[:], zg[:], AF.Sigmoid,
                                                 bias=boutg_sb[:, j:j + 1],
                                                 scale=1.0 / WSCALE)
                            for b in range(BL):
                                scr = sgp.tile([128, 4, M], F16, tag="scr")
                                nc.vector.scalar_tensor_tensor(
                                    scr[:],
                                    za[:, :, b:b + 1, :].squeeze(),
                                    bouta_sb[:, j:j + 1],
                                    sg[:, :, b:b + 1, :].squeeze(),
                                    op0=ALU.add, op1=ALU.mult,
                                    accum_out=m1[:, j:j + 1, b:b + 1,
                                                 tg:tg + 1].squeeze()
                                    .unsqueeze(1))

                if DEBUG_TAPS:
                    nc.sync.dma_start(d_dbg_m1[:], m1[:])
                # ---- decode ----
                with tc.tile_pool(name="dps", bufs=1, space="PSUM") as dpsp:
                    r1 = m1pool.tile([128, 4, BL], F32)
                    nc.vector.reduce_sum(r1[:], m1[:], axis=mybir.AxisListType.X)
                    r2 = m1pool.tile([128, BL, 4], F32)
                    for b in range(BL):
                        nc.vector.tensor_mul(
                            r2[:, b:b + 1, :].squeeze(),
                            r1[:, :, b:b + 1].squeeze(), wdec_sb[:])
                    r3 = m1pool.tile([128, BL], F32)
                    nc.vector.reduce_sum(r3[:], r2[:], axis=mybir.AxisListType.X)
                    ones = m1pool.tile([128, 1], F32)
                    nc.vector.memset(ones[:], 1.0)
                    dp = dpsp.tile([1, 2], F32)
                    nc.tensor.matmul(dp[:], ones[:], r3[:], start=True, stop=True)
                    osb = m1pool.tile([1, 2], F32)
                    nc.vector.tensor_scalar_add(osb[:], dp[:], bdec_sb[:, 0:1])
                    nc.sync.dma_start(d_out[:], osb[:])

    nc.compile()
    return nc


def _get_compiled():
    global _compiled
    if _compiled is None:
        _compiled = _build()
    return _compiled


def _run(inputs, trace=False, **kw):
    aux, u_bds = _prep(inputs)
    nc = _get_compiled()
    in_maps = []
    for cid in range(NCORES):
        m = dict(aux)
        m["u_bd"] = u_bds[cid]
        in_maps.append(m)
    return bass_utils.run_bass_kernel_spmd(
        nc, in_maps, core_ids=list(range(NCORES)), trace=trace, **kw)


def kernel(**inputs):
    inputs = {k: np.asarray(v) for k, v in inputs.items()}
    res = _run(inputs)
    out = np.empty((B, 1), np.float32)
    for cid in range(NCORES):
        out[BL * cid:BL * cid + BL, 0] = res.results[cid]["odec"][0, :]
    return out


# revision 4
# speedup vs baseline: 2.3245x; 1.1981x over previous
"""MiniS4D Trainium2 kernel v3 (8 NeuronCores, batch-data-parallel).

v2 -> v3: wave-batched DMA loads issued from the Sync engine (the v2
Activation engine spent 321us dispatching 528 per-group DMAs), single
masked tensor_tensor_scan per wave (was 8), fp8(e4m3) y + DoubleRow
fp8 mix matmuls (2x PE throughput, K=256 per instruction).

Structure per group g of 4 channels (c = 4g+q), partition p = 32q+t':
  intra: y[(q,t),(b,m)] = sum_{t'<=t} keff32[c,t-t'] u[b,c,32m+t']   (BD toep)
  Q[(q,nh),(b,m)]       = Re/Im sum_{t'} r^{t'} u[b,c,32m+t']        (BD v2)
  Qt = E (.) Q  (complex, E[m] = e^m, e = r^32)   [DVE shuffle+mul+add]
  PP = segmented-cumsum_m Qt  (mask resets at m=0) [one tensor_tensor_scan]
  Hs[m] = K[m] (.) PP[m-1]  (K[m] = r^{L-1-32m}, K[0] := 0)  [DVE]
  y += v1BD^T Hs                                  (BD v1, same PSUM)
GELU -> fp8 -> SBUF-to-SBUF DMA gather to c-major Ybig ->
DoubleRow fp8 W_out mix (W pre-scaled by 32) -> GLU -> mean -> dec.
"""
import sys
sys.path.insert(0, "/opt/trn_rl_repo")
import numpy as np
import ml_dtypes

import concourse.bass as bass
import concourse.tile as tile
from concourse import bacc, mybir
from concourse import bass_utils
from concourse.bass import broadcast_tensor_aps

F32 = mybir.dt.float32
F16 = mybir.dt.float16
F8 = mybir.dt.float8e4
NP8 = ml_dtypes.float8_e4m3fn
AF = mybir.ActivationFunctionType
ALU = mybir.AluOpType
PM = mybir.MatmulPerfMode
GELU_FUNC = None   # set to AF.Copy for CoreSim (no Gelu there)
DEBUG_TAPS = False
WSCALE = 32.0      # fp8 weight pre-scale

B, C, L, N = 16, 512, 4096, 8
T, M, G = 32, 128, 128
NW = 16            # waves of 8 groups
NCORES, BL = 8, 2

SHUF = [(i % 16 + 8) % 16 + 16 * (i // 16) for i in range(32)]

_compiled = None


def _prep(inputs):
    """Host-side parameter preparation (numpy, float64 internally)."""
    log_dt = inputs["log_dt"].astype(np.float64)
    A = -np.exp(inputs["log_A_real"].astype(np.float64)) \
        + 1j * inputs["A_imag"].astype(np.float64)
    dt = np.exp(log_dt)
    r = np.exp(dt[:, None] * A)                                   # (C, N)
    Bc = inputs["B_re"].astype(np.float64) + 1j * inputs["B_im"].astype(np.float64)
    Cc = inputs["C_re"].astype(np.float64) + 1j * inputs["C_im"].astype(np.float64)
    w = Cc * (r - 1.0) / A * Bc                                   # (C, N)
    rinv = 1.0 / r
    wL = w * r ** (L - 1)

    lags = np.arange(T)
    keff32 = np.real(wL[:, :, None] * rinv[:, :, None] ** lags).sum(1)  # (C, 32)
    keff32[:, 0] += inputs["D"].astype(np.float64)

    gidx = np.arange(G)
    toepbd = np.zeros((G, 128, 128), np.float16)
    v2bd = np.zeros((G, 128, 64), np.float16)
    v1bd = np.zeros((G, 64, 128), np.float16)
    rp = r[:, None, :] ** lags[:, None]                           # (C, 32, N)
    wr = w[:, None, :] * rinv[:, None, :] ** lags[:, None]        # (C, 32, N)
    for q in range(4):
        cs = 4 * gidx + q
        blk = np.zeros((G, 32, 32), np.float64)
        for d in range(T):
            idx = np.arange(T - d)
            blk[:, idx, idx + d] = keff32[cs, d][:, None]
        toepbd[:, 32 * q:32 * q + 32, 32 * q:32 * q + 32] = blk.astype(np.float16)
        v2bd[:, 32 * q:32 * q + 32, 16 * q:16 * q + 8] = rp[cs].real
        v2bd[:, 32 * q:32 * q + 32, 16 * q + 8:16 * q + 16] = rp[cs].imag
        v1bd[:, 16 * q:16 * q + 8, 32 * q:32 * q + 32] = \
            wr[cs].real.transpose(0, 2, 1)
        v1bd[:, 16 * q + 8:16 * q + 16, 32 * q:32 * q + 32] = \
            -wr[cs].imag.transpose(0, 2, 1)

    ms = np.arange(M)
    Em = (r ** T)[:, :, None] ** ms                               # (C, N, M)
    Km = r[:, :, None] ** (L - 1 - T * ms)                        # (C, N, M)
    Km[:, :, 0] = 0.0          # Hs[0] = 0 via scale (flat-scan shift trick)
    E1 = np.concatenate([Em.real, Em.real], 1).astype(np.float16)  # (C,16,M)
    E2 = np.concatenate([-Em.imag, Em.imag], 1).astype(np.float16)
    K1 = np.concatenate([Km.real, Km.real], 1).astype(np.float16)
    K2 = np.concatenate([-Km.imag, Km.imag], 1).astype(np.float16)

    # esc[w, p=64*gh+16*q+nh, a, j, m], c = 4*(8w+2j+gh)+q
    esc = np.zeros((NW, 128, 4, 4, M), np.float16)
    warr = np.arange(NW)[:, None]
    jarr = np.arange(4)[None, :]
    for gh in range(2):
        for q in range(4):
            cs = 4 * (8 * warr + 2 * jarr + gh) + q               # (NW, 4)
            p0 = 64 * gh + 16 * q
            for a, S in enumerate((E1, E2, K1, K2)):
                esc[:, p0:p0 + 16, a, :, :] = S[cs].transpose(0, 2, 1, 3)

    # fp8 DoubleRow mix weights: w8[ip, p, i2, o] = WSCALE*W_out[o, 128*(2ip+i2)+p]
    wt = (inputs["W_out"].T.astype(np.float64) * WSCALE).reshape(2, 2, 128, 1024)
    w8 = np.ascontiguousarray(wt.transpose(0, 2, 1, 3)).astype(NP8)  # (2,128,2,1024)
    b_out = inputs["b_out"].astype(np.float32)
    bouta = np.ascontiguousarray(b_out[:512].reshape(4, 128).T) * WSCALE
    boutg = np.ascontiguousarray(b_out[512:].reshape(4, 128).T)
    wdecp = np.ascontiguousarray(
        (inputs["W_dec"][0].astype(np.float32) / (L * WSCALE)).reshape(4, 128).T)
    bdec = inputs["b_dec"].astype(np.float32).reshape(1, 1)

    # p-major layouts: [128, G, ...] so each partition's wave-window is one
    # contiguous DMA descriptor.
    toeppm = np.ascontiguousarray(toepbd.transpose(1, 0, 2))      # (128, G, 128)
    v2pm = np.ascontiguousarray(v2bd.transpose(1, 0, 2))          # (128, G, 64)
    aux = dict(toeppm=toeppm, v2pm=v2pm, v1bd=v1bd, esc=esc, w8=w8,
               bouta=bouta.astype(np.float32), boutg=boutg, wdecp=wdecp,
               bdec=bdec)

    # u_bd per core, p-major: [128(32q+t'), G, BL, M]
    u16 = inputs["u"].astype(np.float16)                          # (B, C, L)
    u_bds = []
    for k in range(NCORES):
        ub = u16[BL * k:BL * k + BL].reshape(BL, G, 4, M, T)
        u_bds.append(np.ascontiguousarray(
            ub.transpose(2, 4, 1, 0, 3).reshape(128, G, BL, M)))
    return aux, u_bds


def _build():
    nc = bacc.Bacc("TRN2", target_bir_lowering=False, debug=False,
                   num_devices=NCORES)
    d_u = nc.dram_tensor("u_bd", [128, G, BL, M], F16, kind="ExternalInput").ap()
    d_toep = nc.dram_tensor("toeppm", [128, G, 128], F16, kind="ExternalInput").ap()
    d_v2 = nc.dram_tensor("v2pm", [128, G, 64], F16, kind="ExternalInput").ap()
    d_v1 = nc.dram_tensor("v1bd", [G, 64, 128], F16, kind="ExternalInput").ap()
    d_esc = nc.dram_tensor("esc", [NW, 128, 4, 4, M], F16, kind="ExternalInput").ap()
    d_w8 = nc.dram_tensor("w8", [2, 128, 2, 1024], F8, kind="ExternalInput").ap()
    d_bouta = nc.dram_tensor("bouta", [128, 4], F32, kind="ExternalInput").ap()
    d_boutg = nc.dram_tensor("boutg", [128, 4], F32, kind="ExternalInput").ap()
    d_wdecp = nc.dram_tensor("wdecp", [128, 4], F32, kind="ExternalInput").ap()
    d_bdec = nc.dram_tensor("bdec", [1, 1], F32, kind="ExternalInput").ap()
    d_out = nc.dram_tensor("odec", [1, 2], F32, kind="ExternalOutput").ap()
    if DEBUG_TAPS:
        d_dbg_ht = nc.dram_tensor("dbg_ht", [128, 4, BL, M], F16,
                                  kind="ExternalOutput").ap()
        d_dbg_y = nc.dram_tensor("dbg_y", [128, 4, 32, BL, M], F8,
                                 kind="ExternalOutput").ap()
        d_dbg_m1 = nc.dram_tensor("dbg_m1", [128, 4, BL, 8], F32,
                                  kind="ExternalOutput").ap()

    with tile.TileContext(nc) as tc:
        with tc.tile_pool(name="const", bufs=1) as constp:
            bouta_sb = constp.tile([128, 4], F32)
            nc.sync.dma_start(bouta_sb[:], d_bouta[:])
            boutg_sb = constp.tile([128, 4], F32)
            nc.sync.dma_start(boutg_sb[:], d_boutg[:])
            wdec_sb = constp.tile([128, 4], F32)
            nc.sync.dma_start(wdec_sb[:], d_wdecp[:])
            bdec_sb = constp.tile([1, 1], F32)
            nc.sync.dma_start(bdec_sb[:], d_bdec[:])
            maskt = constp.tile([128, 4, BL, M], F16)
            nc.vector.memset(maskt[:], 1.0)
            nc.vector.memset(maskt[:, :, :, 0:1], 0.0)
            ybig = constp.tile([128, 4, 32, BL, M], F8)
            w8t = []
            for ip in range(2):
                wt8 = constp.tile([128, 2, 1024], F8, tag=f"w8{ip}")
                nc.sync.dma_start(wt8[:], d_w8[ip])
                w8t.append(wt8)

            # ================= SSM phase (16 waves of 8 groups) =========
            with tc.tile_pool(name="uwp", bufs=3) as uwp, \
                 tc.tile_pool(name="twp", bufs=3) as twp, \
                 tc.tile_pool(name="vwp", bufs=3) as vwp, \
                 tc.tile_pool(name="v1p", bufs=10) as v1p, \
                 tc.tile_pool(name="escp", bufs=2) as escp, \
                 tc.tile_pool(name="stp", bufs=3) as stp, \
                 tc.tile_pool(name="ygp", bufs=8) as ygp, \
                 tc.tile_pool(name="qps", bufs=2, space="PSUM") as qpsp, \
                 tc.tile_pool(name="yps", bufs=4, space="PSUM") as ypsp:

                wave_t = {}

                def v2_stage(w):
                    g0 = 8 * w
                    esct = escp.tile([128, 4, 4, M], F16)
                    nc.sync.dma_start(esct[:], d_esc[w])
                    uw = uwp.tile([128, 8, BL, M], F16)
                    nc.sync.dma_start(uw[:], d_u[:, g0:g0 + 8])
                    tw = twp.tile([128, 8, 128], F16)
                    nc.sync.dma_start(tw[:], d_toep[:, g0:g0 + 8])
                    vw = vwp.tile([128, 8, 64], F16)
                    nc.sync.dma_start(vw[:], d_v2[:, g0:g0 + 8])
                    v1w = v1p.tile([128, 4, 128], F16, tag="v1")
                    nc.sync.dma_start(
                        v1w[:], d_v1[g0:g0 + 8].rearrange(
                            "(j gh) r t -> gh r j t", j=4, gh=2))
                    qps = qpsp.tile([128, 4, BL, M], F32)
                    for jg in range(8):
                        j, gh = jg // 2, jg % 2
                        nc.tensor.matmul(
                            qps[64 * gh:64 * gh + 64, j:j + 1].squeeze(),
                            vw[:, jg:jg + 1, :].squeeze(),
                            uw[:, jg:jg + 1].squeeze(),
                            start=True, stop=True)
                    qs = stp.tile([128, 4, BL, M], F16, tag="qs")
                    nc.scalar.activation(qs[:], qps[:], AF.Copy)
                    sh = stp.tile([128, 4, BL, M], F16, tag="sh")
                    nc.vector.stream_shuffle(sh[:], qs[:], SHUF)
                    t1 = stp.tile([128, 4, BL, M], F16, tag="t1")

                    def bmul(dst, src, a):
                        sl = esct[:, a:a + 1, :, :].squeeze().unsqueeze(2)
                        _, e_ap = broadcast_tensor_aps(src, sl)
                        nc.vector.tensor_mul(dst, src, e_ap)

                    bmul(t1[:], qs[:], 0)
                    bmul(sh[:], sh[:], 1)
                    qt = stp.tile([128, 4, BL, M], F16, tag="qt")
                    nc.vector.tensor_add(qt[:], t1[:], sh[:])
                    # segmented flat scan: PP[jbm] with mask reset at m=0
                    ppc = stp.tile([128, 8 + 4 * BL * M], F16, tag="ppc")
                    nc.vector.memset(ppc[:, 0:8], 0.0)
                    nc.vector.tensor_tensor_scan(
                        ppc[:, 8:8 + 4 * BL * M],
                        maskt[:].rearrange("p j b m -> p (j b m)"),
                        qt[:].rearrange("p j b m -> p (j b m)"),
                        0.0, ALU.mult, ALU.add)
                    ppshift = ppc[:, 7:7 + 4 * BL * M].rearrange(
                        "p (j b m) -> p j b m", j=4, b=BL, m=M)
                    psh = stp.tile([128, 4, BL, M], F16, tag="psh")
                    nc.vector.stream_shuffle(
                        psh[:].rearrange("p j b m -> p (j b m)"),
                        ppc[:, 7:7 + 4 * BL * M], SHUF)
                    bmul(t1[:], ppshift, 2)
                    bmul(psh[:], psh[:], 3)
                    ht = stp.tile([128, 4, BL, M], F16, tag="ht")
                    nc.vector.tensor_add(ht[:], t1[:], psh[:])
                    wave_t[w] = (uw, tw, v1w, ht)
                    if DEBUG_TAPS and w == 0:
                        nc.sync.dma_start(d_dbg_ht[:], ht[:])

                def conv_stage(w):
                    uw, tw, v1w, ht = wave_t.pop(w)
                    for jp in range(4):
                        yps = ypsp.tile([128, 2, BL, M], F32)
                        for gg in range(2):
                            jg = 2 * jp + gg
                            j, gh = jg // 2, jg % 2
                            nc.tensor.matmul(
                                yps[:, gg:gg + 1].squeeze(),
                                tw[:, jg:jg + 1, :].squeeze(),
                                uw[:, jg:jg + 1].squeeze(),
                                start=True, stop=False)
                            nc.tensor.matmul(
                                yps[:, gg:gg + 1].squeeze(),
                                v1w[64 * gh:64 * gh + 64, j:j + 1].squeeze(),
                                ht[64 * gh:64 * gh + 64, j:j + 1].squeeze(),
                                start=False, stop=True)
                        yg = ygp.tile([128, 2, BL, M], F8, tag="yg")
                        nc.scalar.activation(yg[:], yps[:], GELU_FUNC or AF.Gelu)
                        for gg in range(2):
                            g = 8 * w + 2 * jp + gg
                            ghat, i = g % 32, g // 32
                            eng = nc.sync if gg == 0 else nc.scalar
                            eng.dma_start(
                                ybig[4 * ghat:4 * ghat + 4, i:i + 1].squeeze(),
                                yg[:, gg:gg + 1].squeeze())

                for w in range(NW + 1):
                    if w < NW:
                        v2_stage(w)
                    if w >= 1:
                        conv_stage(w - 1)

            if DEBUG_TAPS:
                nc.sync.dma_start(d_dbg_y[:], ybig[:])

            # ============ Mix phase (fp8 DoubleRow, K=256/instr) ========
            with tc.tile_pool(name="m1p", bufs=1) as m1pool:
                m1 = m1pool.tile([128, 4, BL, 8], F32)
                with tc.tile_pool(name="sgp", bufs=4) as sgp, \
                     tc.tile_pool(name="zps", bufs=2, space="PSUM") as zpsp:
                    for j in range(4):
                        for tg in range(8):
                            za = zpsp.tile([128, 4, BL, M], F32, tag="za")
                            zg = zpsp.tile([128, 4, BL, M], F32, tag="zg")
                            for side, zt in ((0, za), (1, zg)):
                                o = j + 4 * side
                                for ip in range(2):
                                    for hw in range(2):
                                        t0 = 4 * tg + 2 * hw
                                        nc.tensor.matmul(
                                            zt[:, 2 * hw:2 * hw + 2],
                                            w8t[ip][:, :,
                                                    128 * o:128 * o + 128],
                                            ybig[:, 2 * ip:2 * ip + 2,
                                                 t0:t0 + 2],
                                            start=(ip == 0), stop=(ip == 1),
                                            perf_mode=PM.DoubleRow)
                            sg = sgp.tile([128, 4, BL, M], F16, tag="sg")
                            nc.scalar.activation(sg[:], zg[:], AF.Sigmoid,
                                                 bias=boutg_sb[:, j:j + 1],
                                                 scale=1.0 / WSCALE)
                            zaf = sgp.tile([128, 4, BL, M], F16, tag="zaf")
                            nc.scalar.activation(zaf[:], za[:], AF.Copy)
                            for b in range(BL):
                                scr = sgp.tile([128, 4, M], F16, tag="scr")
                                nc.vector.scalar_tensor_tensor(
                                    scr[:],
                                    zaf[:, :, b:b + 1, :].squeeze(),
                                    bouta_sb[:, j:j + 1],
                                    sg[:, :, b:b + 1, :].squeeze(),
                                    op0=ALU.add, op1=ALU.mult,
                                    accum_out=m1[:, j:j + 1, b:b + 1,
                                                 tg:tg + 1].squeeze()
                                    .unsqueeze(1))

                if DEBUG_TAPS:
                    nc.sync.dma_start(d_dbg_m1[:], m1[:])
                # ---- decode ----
                with tc.tile_pool(name="dps", bufs=1, space="PSUM") as dpsp:
                    r1 = m1pool.tile([128, 4, BL], F32)
                    nc.vector.reduce_sum(r1[:], m1[:], axis=mybir.AxisListType.X)
                    r2 = m1pool.tile([128, BL, 4], F32)
                    for b in range(BL):
                        nc.vector.tensor_mul(
                            r2[:, b:b + 1, :].squeeze(),
                            r1[:, :, b:b + 1].squeeze(), wdec_sb[:])
                    r3 = m1pool.tile([128, BL], F32)
                    nc.vector.reduce_sum(r3[:], r2[:], axis=mybir.AxisListType.X)
                    ones = m1pool.tile([128, 1], F32)
                    nc.vector.memset(ones[:], 1.0)
                    dp = dpsp.tile([1, 2], F32)
                    nc.tensor.matmul(dp[:], ones[:], r3[:], start=True, stop=True)
                    osb = m1pool.tile([1, 2], F32)
                    nc.vector.tensor_scalar_add(osb[:], dp[:], bdec_sb[:, 0:1])
                    nc.sync.dma_start(d_out[:], osb[:])

    nc.compile()
    return nc


def _get_compiled():
    global _compiled
    if _compiled is None:
        _compiled = _build()
    return _compiled


def _run(inputs, trace=False, **kw):
    aux, u_bds = _prep(inputs)
    nc = _get_compiled()
    in_maps = []
    for cid in range(NCORES):
        m = dict(aux)
        m["u_bd"] = u_bds[cid]
        in_maps.append(m)
    return bass_utils.run_bass_kernel_spmd(
        nc, in_maps, core_ids=list(range(NCORES)), trace=trace, **kw)


def kernel(**inputs):
    inputs = {k: np.asarray(v) for k, v in inputs.items()}
    res = _run(inputs)
    out = np.empty((B, 1), np.float32)
    for cid in range(NCORES):
        out[BL * cid:BL * cid + BL, 0] = res.results[cid]["odec"][0, :]
    return out
